# revision 16
# baseline (speedup 1.0000x reference)
# Trainium2 Bass kernel for EndPointRepr (span endpoint representations).
#
# reference:
#   h = encoded_input @ W + b                    # [B, S, P]
#   res_k[q] = concat(h[qb[q], s_k[q]], h[qb[q], e_k[q]]) * (e_k[q] >= s_k[q])
#
# Sharding: data-parallel over batch. Core c owns batch c; the host routes
# each valid (e >= s) query to its batch's core. Invalid queries are never
# routed; the host-side result buffers start zeroed.
#
# Device pipeline (bf16 data path, fp32 PSUM accumulation):
#   The host compacts the batch to the h rows actually referenced (~1350 of
#   2048, capacity HROWS, ascending order) and remaps indices. Each of the
#   four endpoint streams (s1, e1, s2, e2) gets its OWN slot order sorted by
#   referenced row, so a 128-slot tile only touches a ~3-block band of h.
#   The host reassembles res from the four streams independently.
#   phase 1: X chunks stream in, k-halves split across the two HWDGE queues;
#            per 128-row block, 8 k-block matmuls accumulate h in PSUM; DVE
#            folds the bias while down-casting to bf16 SBUF tiles that stay
#            resident. Identity warmup matmuls ramp the PE p-state.
#   phase 2: gather AS MATMUL: per (stream, tile), host-built one-hot
#            selection matrices (exact 1.0 entries) multiply the resident h
#            blocks: res_tile[q, :] = sum_kb onehot[kb][:, q].T @ h[kb].
#            ACT/DVE copy PSUM to bf16 and plain DMAs write the result.
#            No GpSimd ucode, no DRAM h scratch, no indirect DMA; row
#            selection by 1.0-matmul is numerically exact.
# bf16 keeps the PE at 1 cycle/row (fp32 is 4) and halves all DMA traffic;
# rel err ~3e-3 against the fp32 reference, well inside the 2e-2 gate.
import numpy as np

B, S, D, P = 8, 2048, 1024, 256
NQ = 8192
NCORES = 8
KB = D // 128          # contraction k-blocks
HROWS = 1536           # compacted h row capacity (multiple of SCHW)
HB = HROWS // 128      # h row blocks
QCAP = 640             # per-endpoint query capacity (multiple of 128)
QT = QCAP // 128       # query tiles per endpoint stream
NST = 4                # endpoint streams: s1, e1, s2, e2
SCH = 4                # x chunks for DMA/matmul overlap
SCHW = HROWS // SCH
NWARM = 32             # PE warmup matmuls (p-state ramp during preamble)

_cache = {}


def _build_nc():
    import concourse.bacc as bacc
    import concourse.mybir as mybir
    import concourse.tile as tile
    from concourse.masks import make_identity

    f32 = mybir.dt.float32
    bf16 = mybir.dt.bfloat16
    nc = bacc.Bacc("TRN2", target_bir_lowering=False, debug=False,
                   num_devices=NCORES)

    # per-(stream, tile) k-block window [kbase, kbase+kcnt) over h blocks
    kwin = _cache["kwin"]                  # list of NST*QT (kbase, kcnt)
    bases = np.cumsum([0] + [kc for _, kc in kwin]).tolist()
    noh = bases[-1]                        # total one-hot [128,128] tiles

    xh = nc.dram_tensor("xh", [128, SCH * KB * SCHW], bf16,
                        kind="ExternalInput").ap()
    wh = nc.dram_tensor("wh", [128, KB * P], bf16, kind="ExternalInput").ap()
    bias = nc.dram_tensor("bias", [128, P], f32, kind="ExternalInput").ap()
    oh = nc.dram_tensor("oh", [128, noh * 128], bf16,
                        kind="ExternalInput").ap()
    r1 = nc.dram_tensor("r1", [QCAP, 2 * P], bf16, kind="ExternalOutput").ap()
    r2 = nc.dram_tensor("r2", [QCAP, 2 * P], bf16, kind="ExternalOutput").ap()

    with tile.TileContext(nc) as tc:
        with (
            tc.tile_pool(name="consts", bufs=1) as consts,
            tc.tile_pool(name="xin", bufs=SCH) as xt_pool,
            tc.tile_pool(name="gout", bufs=6) as g_pool,
            tc.tile_pool(name="ps", bufs=4, space="PSUM") as ps_pool,
            tc.tile_pool(name="psg", bufs=4, space="PSUM") as psg_pool,
        ):
            identity = consts.tile([128, 128], bf16)
            make_identity(nc, identity)
            for i in range(NWARM):
                warm_ps = psg_pool.tile([128, 128], f32, tag="gps")
                nc.tensor.matmul(warm_ps, identity, identity,
                                 start=True, stop=True)

            w_sb = consts.tile([128, KB, P], bf16)
            nc.scalar.dma_start(w_sb,
                                wh.rearrange("p (kb j) -> p kb j", kb=KB))
            bias_sb = consts.tile([128, P], f32)
            nc.scalar.dma_start(bias_sb, bias)

            # x chunk loads, k-thirds split across three queues, up-front
            xh_view = xh.rearrange("p (c kb s) -> p c kb s", c=SCH, kb=KB)
            xt_tiles = [xt_pool.tile([128, KB, SCHW], bf16, name=f"xt{c}")
                        for c in range(SCH)]
            qs = [nc.sync, nc.scalar, nc.gpsimd]
            ksplit = [(0, 3), (3, 6), (6, KB)]
            for qi, (k0, k1) in enumerate(ksplit):
                for c in range(SCH):
                    qs[qi].dma_start(xt_tiles[c][:, k0:k1, :],
                                     xh_view[:, c, k0:k1, :])
            # one-hot tiles, thirds on each queue, behind the x loads
            oh_sb = consts.tile([128, noh, 128], bf16)
            oh_view = oh.rearrange("p (i q) -> p i q", q=128)
            n3 = noh // 3
            osplit = [(0, n3), (n3, 2 * n3), (2 * n3, noh)]
            for qi, (o0, o1) in enumerate(osplit):
                if o1 > o0:
                    qs[qi].dma_start(oh_sb[:, o0:o1, :], oh_view[:, o0:o1, :])

            # phase 1: h = X @ W + b, blocks stay resident in SBUF
            h_tiles = []
            for c in range(SCH):
                for ml in range(SCHW // 128):
                    m = c * (SCHW // 128) + ml
                    h_ps = ps_pool.tile([128, P], f32, tag="hps")
                    for kb in range(KB):
                        nc.tensor.matmul(
                            h_ps,
                            xt_tiles[c][:, kb, ml * 128:(ml + 1) * 128],
                            w_sb[:, kb, :],
                            start=(kb == 0), stop=(kb == KB - 1))
                    h_sb = consts.tile([128, P], bf16, name=f"h{m}")
                    nc.vector.tensor_add(h_sb, h_ps, bias_sb)
                    h_tiles.append(h_sb)

            # phase 2: gather as one-hot matmuls
            for st in range(NST):
                r = r1 if st < 2 else r2
                endp = st % 2
                out_view = r.rearrange("(t p) c -> p t c", p=128)
                for t in range(QT):
                    j = st * QT + t
                    kbase, kcnt = kwin[j]
                    g_ps = psg_pool.tile([128, P], f32, tag="gps")
                    for l in range(kcnt):
                        nc.tensor.matmul(
                            g_ps, oh_sb[:, bases[j] + l, :],
                            h_tiles[kbase + l],
                            start=(l == 0), stop=(l == kcnt - 1))
                    g_sb = g_pool.tile([128, 1, P], bf16, tag="g")
                    if (st * QT + t) % 2 == 0:
                        nc.vector.tensor_copy(g_sb[:, 0, :], g_ps)
                    else:
                        nc.scalar.copy(g_sb[:, 0, :], g_ps)
                    eng = qs[(st * QT + t) % 3]
                    eng.dma_start(
                        out_view[:, t:t + 1, endp * P:(endp + 1) * P],
                        g_sb)

    nc.compile()
    return nc


def _get_nc(kwin):
    key = ("nc", tuple(kwin))
    if key not in _cache:
        _cache["kwin"] = list(kwin)
        _cache[key] = _build_nc()
    return _cache[key]


def _numpy_ref(flag, encoded_input, start_ids_1, end_ids_1, query_batch_idx,
               start_ids_2, end_ids_2, W, b):
    h = encoded_input.astype(np.float32) @ W.astype(np.float32) + \
        b.astype(np.float32)
    qb = np.asarray(query_batch_idx).astype(np.int64)

    def span(s, e):
        s = np.asarray(s).astype(np.int64)
        e = np.asarray(e).astype(np.int64)
        rep = np.concatenate([h[qb, s], h[qb, e]], axis=-1)
        return rep * (e >= s)[:, None].astype(rep.dtype)

    return span(start_ids_1, end_ids_1), span(start_ids_2, end_ids_2)


def kernel(flag, encoded_input, start_ids_1, end_ids_1, query_batch_idx,
           start_ids_2, end_ids_2, W, b):
    import ml_dtypes
    from concourse.bass_utils import run_bass_kernel_spmd

    bf16 = ml_dtypes.bfloat16
    x_full = np.asarray(encoded_input, dtype=np.float32)
    w_np = np.asarray(W, dtype=np.float32)
    b_np = np.asarray(b).astype(np.float32)
    qb = np.asarray(query_batch_idx).astype(np.int64)
    s1 = np.asarray(start_ids_1).astype(np.int64)
    e1 = np.asarray(end_ids_1).astype(np.int64)
    s2 = np.asarray(start_ids_2).astype(np.int64)
    e2 = np.asarray(end_ids_2).astype(np.int64)

    in_range = (qb.min() >= 0 and qb.max() < B and
                all(a.min() >= 0 and a.max() < S for a in (s1, e1, s2, e2)))

    percore = []
    try:
        if not in_range or x_full.shape != (B, S, D):
            raise ValueError("shape/range")
        for bb in range(B):
            sel = qb == bb
            ids1 = np.nonzero(sel & (e1 >= s1))[0]
            ids2 = np.nonzero(sel & (e2 >= s2))[0]
            if len(ids1) > QCAP or len(ids2) > QCAP:
                raise ValueError("capacity overflow")
            rows = np.unique(np.concatenate(
                [s1[ids1], e1[ids1], s2[ids2], e2[ids2]]))
            if len(rows) > HROWS:
                raise ValueError("row overflow")
            # per endpoint-stream: slot ids sorted by referenced row
            streams = []
            for ids, a in [(ids1, s1), (ids1, e1), (ids2, s2), (ids2, e2)]:
                cr = np.searchsorted(rows, a[ids]).astype(np.int64)
                o = np.argsort(cr, kind="stable")
                streams.append((ids[o], cr[o]))
            percore.append((rows, streams))
        # merged k-windows per (stream, tile) across cores
        kwin = []
        for st in range(NST):
            for t in range(QT):
                lo, hi = HB - 1, 0
                for bb in range(B):
                    cr = percore[bb][1][st][1]
                    seg = cr[t * 128:(t + 1) * 128]
                    if len(seg):
                        lo = min(lo, int(seg[0]) // 128)
                        hi = max(hi, int(seg[-1]) // 128)
                if hi < lo:
                    lo, hi = 0, 0
                kwin.append((lo, hi - lo + 1))
        bases = np.cumsum([0] + [kc for _, kc in kwin])
        noh = int(bases[-1])

        wh = np.ascontiguousarray(
            w_np.reshape(KB, 128, P).transpose(1, 0, 2).reshape(128, KB * P)
        ).astype(bf16)
        bias_rep = np.ascontiguousarray(
            np.broadcast_to(b_np[None, :], (128, P)), dtype=np.float32)
        in_maps, ids_all = [], []
        for bb in range(B):
            rows, streams = percore[bb]
            ids_all.append(streams)
            oh_np = np.zeros((128, noh, 128), np.float32)
            for st in range(NST):
                ids, cr = streams[st]
                n = len(ids)
                for t in range(QT):
                    j = st * QT + t
                    kbase, kcnt = kwin[j]
                    seg = cr[t * 128:min(n, (t + 1) * 128)]
                    q = np.arange(len(seg))
                    oh_np[seg % 128, bases[j] + seg // 128 - kbase, q] = 1.0
            xc = np.zeros((HROWS, D), np.float32)
            xc[:len(rows)] = x_full[bb][rows]
            xr = xc.reshape(SCH, SCHW, KB, 128).transpose(3, 0, 2, 1) \
                .reshape(128, SCH * KB * SCHW)
            in_maps.append({
                "xh": np.ascontiguousarray(xr).astype(bf16),
                "wh": wh,
                "bias": bias_rep,
                "oh": np.ascontiguousarray(
                    oh_np.reshape(128, noh * 128)).astype(bf16),
            })
    except ValueError:
        res1, res2 = _numpy_ref(flag, x_full, s1, e1, qb, s2, e2, w_np, b_np)
        return np.asarray(res1, np.float32), np.asarray(res2, np.float32)

    nc = _get_nc(tuple(kwin))
    out = run_bass_kernel_spmd(nc, in_maps, core_ids=list(range(NCORES)))
    _cache["last_run"] = out

    res1 = np.zeros((NQ, 2 * P), np.float32)
    res2 = np.zeros((NQ, 2 * P), np.float32)
    for bb in range(B):
        streams = ids_all[bb]
        rr1 = np.asarray(out.results[bb]["r1"]).astype(np.float32)
        rr2 = np.asarray(out.results[bb]["r2"]).astype(np.float32)
        for st, (res, rr) in enumerate([(res1, rr1), (res1, rr1),
                                        (res2, rr2), (res2, rr2)]):
            ids, _ = streams[st]
            endp = st % 2
            n = len(ids)
            if n:
                res[ids, endp * P:(endp + 1) * P] = \
                    rr[:n, endp * P:(endp + 1) * P]
    return res1, res2


# revision 22
# speedup vs baseline: 1.0357x; 1.0357x over previous
# Trainium2 Bass kernel for EndPointRepr (span endpoint representations).
#
# reference:
#   h = encoded_input @ W + b                    # [B, S, P]
#   res_k[q] = concat(h[qb[q], s_k[q]], h[qb[q], e_k[q]]) * (e_k[q] >= s_k[q])
#
# Sharding: data-parallel over batch. Core c owns batch c; the host routes
# each valid (e >= s) query to its batch's core. Invalid queries are never
# routed; the host-side result buffers start zeroed.
#
# Device pipeline (bf16 data path, fp32 PSUM accumulation):
#   The host compacts the batch to the h rows actually referenced (~1350 of
#   2048, capacity HROWS, ascending order) and remaps indices. Each of the
#   four endpoint streams (s1, e1, s2, e2) gets its OWN slot order sorted by
#   referenced row, so a 128-slot tile only touches a ~3-block band of h.
#   The host reassembles res from the four streams independently.
#   phase 1: X chunks stream in, k-halves split across the two HWDGE queues;
#            per 128-row block, 8 k-block matmuls accumulate h in PSUM; DVE
#            folds the bias while down-casting to bf16 SBUF tiles that stay
#            resident. Identity warmup matmuls ramp the PE p-state.
#   phase 2: gather AS MATMUL: per (stream, tile), host-built one-hot
#            selection matrices (exact 1.0 entries) multiply the resident h
#            blocks: res_tile[q, :] = sum_kb onehot[kb][:, q].T @ h[kb].
#            ACT/DVE copy PSUM to bf16 and plain DMAs write the result.
#            No GpSimd ucode, no DRAM h scratch, no indirect DMA; row
#            selection by 1.0-matmul is numerically exact.
# bf16 keeps the PE at 1 cycle/row (fp32 is 4) and halves all DMA traffic;
# rel err ~3e-3 against the fp32 reference, well inside the 2e-2 gate.
import numpy as np

B, S, D, P = 8, 2048, 1024, 256
NQ = 8192
NCORES = 8
KB = D // 128          # contraction k-blocks
HROWS = 1536           # compacted h row capacity (multiple of SCHW)
HB = HROWS // 128      # h row blocks
QCAP = 640             # per-endpoint query capacity (multiple of 128)
QT = QCAP // 128       # query tiles per endpoint stream
NST = 4                # endpoint streams: s1, e1, s2, e2
SCH = 4                # x chunks for DMA/matmul overlap
SCHW = HROWS // SCH
NWARM = 32             # PE warmup matmuls (p-state ramp during preamble)

_cache = {}


def _build_nc():
    import concourse.bacc as bacc
    import concourse.mybir as mybir
    import concourse.tile as tile
    from concourse.masks import make_identity

    f32 = mybir.dt.float32
    bf16 = mybir.dt.bfloat16
    nc = bacc.Bacc("TRN2", target_bir_lowering=False, debug=False,
                   num_devices=NCORES)

    # per-(stream, tile) k-block window [kbase, kbase+kcnt) over h blocks
    kwin = _cache["kwin"]                  # list of NST*QT (kbase, kcnt)
    bases = np.cumsum([0] + [kc for _, kc in kwin]).tolist()
    noh = bases[-1]                        # total one-hot [128,128] tiles

    xh = nc.dram_tensor("xh", [128, SCH * KB * SCHW], bf16,
                        kind="ExternalInput").ap()
    wh = nc.dram_tensor("wh", [128, KB * P], bf16, kind="ExternalInput").ap()
    bias = nc.dram_tensor("bias", [128, P], f32, kind="ExternalInput").ap()
    oh = nc.dram_tensor("oh", [128, noh * 128], bf16,
                        kind="ExternalInput").ap()
    r1 = nc.dram_tensor("r1", [QCAP, 2 * P], bf16, kind="ExternalOutput").ap()
    r2 = nc.dram_tensor("r2", [QCAP, 2 * P], bf16, kind="ExternalOutput").ap()

    with tile.TileContext(nc) as tc:
        with (
            tc.tile_pool(name="consts", bufs=1) as consts,
            tc.tile_pool(name="xin", bufs=SCH) as xt_pool,
            tc.tile_pool(name="gout", bufs=6) as g_pool,
            tc.tile_pool(name="ps", bufs=4, space="PSUM") as ps_pool,
            tc.tile_pool(name="psg", bufs=4, space="PSUM") as psg_pool,
        ):
            identity = consts.tile([128, 128], bf16)
            make_identity(nc, identity)
            for i in range(NWARM):
                warm_ps = psg_pool.tile([128, 128], f32, tag="gps")
                nc.tensor.matmul(warm_ps, identity, identity,
                                 start=True, stop=True)

            w_sb = consts.tile([128, KB, P], bf16)
            nc.scalar.dma_start(w_sb,
                                wh.rearrange("p (kb j) -> p kb j", kb=KB))
            bias_sb = consts.tile([128, P], f32)
            nc.scalar.dma_start(bias_sb, bias)

            # x chunk loads, k-halves split across the two queues, up-front
            xh_view = xh.rearrange("p (c kb s) -> p c kb s", c=SCH, kb=KB)
            KH = KB // 2
            xt_tiles = [xt_pool.tile([128, KB, SCHW], bf16, name=f"xt{c}")
                        for c in range(SCH)]
            for c in range(SCH):
                nc.sync.dma_start(xt_tiles[c][:, 0:KH, :],
                                  xh_view[:, c, 0:KH, :])
            for c in range(SCH):
                nc.scalar.dma_start(xt_tiles[c][:, KH:KB, :],
                                    xh_view[:, c, KH:KB, :])
            # one-hot tiles, halves on each queue, behind the x loads
            oh_sb = consts.tile([128, noh, 128], bf16)
            oh_view = oh.rearrange("p (i q) -> p i q", q=128)
            nh = noh // 2
            nc.sync.dma_start(oh_sb[:, 0:nh, :], oh_view[:, 0:nh, :])
            nc.scalar.dma_start(oh_sb[:, nh:noh, :], oh_view[:, nh:noh, :])

            # phase 1: h = X @ W + b, blocks stay resident in SBUF
            h_tiles = []
            for c in range(SCH):
                for ml in range(SCHW // 128):
                    m = c * (SCHW // 128) + ml
                    h_ps = ps_pool.tile([128, P], f32, tag="hps")
                    for kb in range(KB):
                        nc.tensor.matmul(
                            h_ps,
                            xt_tiles[c][:, kb, ml * 128:(ml + 1) * 128],
                            w_sb[:, kb, :],
                            start=(kb == 0), stop=(kb == KB - 1))
                    h_sb = consts.tile([128, P], bf16, name=f"h{m}")
                    nc.vector.tensor_add(h_sb, h_ps, bias_sb)
                    h_tiles.append(h_sb)

            # phase 2: gather as one-hot matmuls
            for st in range(NST):
                r = r1 if st < 2 else r2
                endp = st % 2
                out_view = r.rearrange("(t p) c -> p t c", p=128)
                for t in range(QT):
                    j = st * QT + t
                    kbase, kcnt = kwin[j]
                    g_ps = psg_pool.tile([128, P], f32, tag="gps")
                    for l in range(kcnt):
                        nc.tensor.matmul(
                            g_ps, oh_sb[:, bases[j] + l, :],
                            h_tiles[kbase + l],
                            start=(l == 0), stop=(l == kcnt - 1))
                    g_sb = g_pool.tile([128, 1, P], bf16, tag="g")
                    if (st * QT + t) % 2 == 0:
                        nc.vector.tensor_copy(g_sb[:, 0, :], g_ps)
                    else:
                        nc.scalar.copy(g_sb[:, 0, :], g_ps)
                    eng = nc.sync if st < 2 else nc.scalar
                    eng.dma_start(
                        out_view[:, t:t + 1, endp * P:(endp + 1) * P],
                        g_sb)

    nc.compile()
    return nc


def _get_nc(kwin):
    key = ("nc", tuple(kwin))
    if key not in _cache:
        _cache["kwin"] = list(kwin)
        _cache[key] = _build_nc()
    return _cache[key]


def _numpy_ref(flag, encoded_input, start_ids_1, end_ids_1, query_batch_idx,
               start_ids_2, end_ids_2, W, b):
    h = encoded_input.astype(np.float32) @ W.astype(np.float32) + \
        b.astype(np.float32)
    qb = np.asarray(query_batch_idx).astype(np.int64)

    def span(s, e):
        s = np.asarray(s).astype(np.int64)
        e = np.asarray(e).astype(np.int64)
        rep = np.concatenate([h[qb, s], h[qb, e]], axis=-1)
        return rep * (e >= s)[:, None].astype(rep.dtype)

    return span(start_ids_1, end_ids_1), span(start_ids_2, end_ids_2)


def kernel(flag, encoded_input, start_ids_1, end_ids_1, query_batch_idx,
           start_ids_2, end_ids_2, W, b):
    import ml_dtypes
    from concourse.bass_utils import run_bass_kernel_spmd

    bf16 = ml_dtypes.bfloat16
    x_full = np.asarray(encoded_input, dtype=np.float32)
    w_np = np.asarray(W, dtype=np.float32)
    b_np = np.asarray(b).astype(np.float32)
    qb = np.asarray(query_batch_idx).astype(np.int64)
    s1 = np.asarray(start_ids_1).astype(np.int64)
    e1 = np.asarray(end_ids_1).astype(np.int64)
    s2 = np.asarray(start_ids_2).astype(np.int64)
    e2 = np.asarray(end_ids_2).astype(np.int64)

    in_range = (qb.min() >= 0 and qb.max() < B and
                all(a.min() >= 0 and a.max() < S for a in (s1, e1, s2, e2)))

    percore = []
    try:
        if not in_range or x_full.shape != (B, S, D):
            raise ValueError("shape/range")
        for bb in range(B):
            sel = qb == bb
            ids1 = np.nonzero(sel & (e1 >= s1))[0]
            ids2 = np.nonzero(sel & (e2 >= s2))[0]
            if len(ids1) > QCAP or len(ids2) > QCAP:
                raise ValueError("capacity overflow")
            rows = np.unique(np.concatenate(
                [s1[ids1], e1[ids1], s2[ids2], e2[ids2]]))
            if len(rows) > HROWS:
                raise ValueError("row overflow")
            # per endpoint-stream: slot ids sorted by referenced row
            streams = []
            for ids, a in [(ids1, s1), (ids1, e1), (ids2, s2), (ids2, e2)]:
                cr = np.searchsorted(rows, a[ids]).astype(np.int64)
                o = np.argsort(cr, kind="stable")
                streams.append((ids[o], cr[o]))
            percore.append((rows, streams))
        # merged k-windows per (stream, tile) across cores
        kwin = []
        for st in range(NST):
            for t in range(QT):
                lo, hi = HB - 1, 0
                for bb in range(B):
                    cr = percore[bb][1][st][1]
                    seg = cr[t * 128:(t + 1) * 128]
                    if len(seg):
                        lo = min(lo, int(seg[0]) // 128)
                        hi = max(hi, int(seg[-1]) // 128)
                if hi < lo:
                    lo, hi = 0, 0
                kwin.append((lo, hi - lo + 1))
        bases = np.cumsum([0] + [kc for _, kc in kwin])
        noh = int(bases[-1])

        wh = np.ascontiguousarray(
            w_np.reshape(KB, 128, P).transpose(1, 0, 2).reshape(128, KB * P)
        ).astype(bf16)
        bias_rep = np.ascontiguousarray(
            np.broadcast_to(b_np[None, :], (128, P)), dtype=np.float32)
        in_maps, ids_all = [], []
        for bb in range(B):
            rows, streams = percore[bb]
            ids_all.append(streams)
            oh_np = np.zeros((128, noh, 128), np.float32)
            for st in range(NST):
                ids, cr = streams[st]
                n = len(ids)
                for t in range(QT):
                    j = st * QT + t
                    kbase, kcnt = kwin[j]
                    seg = cr[t * 128:min(n, (t + 1) * 128)]
                    q = np.arange(len(seg))
                    oh_np[seg % 128, bases[j] + seg // 128 - kbase, q] = 1.0
            xc = np.zeros((HROWS, D), np.float32)
            xc[:len(rows)] = x_full[bb][rows]
            xr = xc.reshape(SCH, SCHW, KB, 128).transpose(3, 0, 2, 1) \
                .reshape(128, SCH * KB * SCHW)
            in_maps.append({
                "xh": np.ascontiguousarray(xr).astype(bf16),
                "wh": wh,
                "bias": bias_rep,
                "oh": np.ascontiguousarray(
                    oh_np.reshape(128, noh * 128)).astype(bf16),
            })
    except ValueError:
        res1, res2 = _numpy_ref(flag, x_full, s1, e1, qb, s2, e2, w_np, b_np)
        return np.asarray(res1, np.float32), np.asarray(res2, np.float32)

    nc = _get_nc(tuple(kwin))
    out = run_bass_kernel_spmd(nc, in_maps, core_ids=list(range(NCORES)))
    _cache["last_run"] = out

    res1 = np.zeros((NQ, 2 * P), np.float32)
    res2 = np.zeros((NQ, 2 * P), np.float32)
    for bb in range(B):
        streams = ids_all[bb]
        rr1 = np.asarray(out.results[bb]["r1"]).astype(np.float32)
        rr2 = np.asarray(out.results[bb]["r2"]).astype(np.float32)
        for st, (res, rr) in enumerate([(res1, rr1), (res1, rr1),
                                        (res2, rr2), (res2, rr2)]):
            ids, _ = streams[st]
            endp = st % 2
            n = len(ids)
            if n:
                res[ids, endp * P:(endp + 1) * P] = \
                    rr[:n, endp * P:(endp + 1) * P]
    return res1, res2


# revision 23
# speedup vs baseline: 1.0454x; 1.0094x over previous
# Trainium2 Bass kernel for EndPointRepr (span endpoint representations).
#
# reference:
#   h = encoded_input @ W + b                    # [B, S, P]
#   res_k[q] = concat(h[qb[q], s_k[q]], h[qb[q], e_k[q]]) * (e_k[q] >= s_k[q])
#
# Sharding: data-parallel over batch. Core c owns batch c; the host routes
# each valid (e >= s) query to its batch's core. Invalid queries are never
# routed; the host-side result buffers start zeroed.
#
# Device pipeline (bf16 data path, fp32 PSUM accumulation):
#   The host compacts the batch to the h rows actually referenced (~1350 of
#   2048, capacity HROWS, ascending order) and remaps indices. Each of the
#   four endpoint streams (s1, e1, s2, e2) gets its OWN slot order sorted by
#   referenced row, so a 128-slot tile only touches a ~3-block band of h.
#   The host reassembles res from the four streams independently.
#   phase 1: X chunks stream in, k-halves split across the two HWDGE queues;
#            per 128-row block, 8 k-block matmuls accumulate h in PSUM; DVE
#            folds the bias while down-casting to bf16 SBUF tiles that stay
#            resident. Identity warmup matmuls ramp the PE p-state.
#   phase 2: gather AS MATMUL: per (stream, tile), host-built one-hot
#            selection matrices (exact 1.0 entries) multiply the resident h
#            blocks: res_tile[q, :] = sum_kb onehot[kb][:, q].T @ h[kb].
#            ACT/DVE copy PSUM to bf16 and plain DMAs write the result.
#            No GpSimd ucode, no DRAM h scratch, no indirect DMA; row
#            selection by 1.0-matmul is numerically exact.
# bf16 keeps the PE at 1 cycle/row (fp32 is 4) and halves all DMA traffic;
# rel err ~3e-3 against the fp32 reference, well inside the 2e-2 gate.
import numpy as np

B, S, D, P = 8, 2048, 1024, 256
NQ = 8192
NCORES = 8
KB = D // 128          # contraction k-blocks
HROWS = 1536           # compacted h row capacity (multiple of SCHW)
HB = HROWS // 128      # h row blocks
QCAP = 640             # per-endpoint query capacity (multiple of 128)
QT = QCAP // 128       # query tiles per endpoint stream
NST = 4                # endpoint streams: s1, e1, s2, e2
SCH = 4                # x chunks for DMA/matmul overlap
SCHW = HROWS // SCH
NWARM = 32             # PE warmup matmuls (p-state ramp during preamble)

_cache = {}


def _build_nc():
    import concourse.bacc as bacc
    import concourse.mybir as mybir
    import concourse.tile as tile
    from concourse.masks import make_identity

    f32 = mybir.dt.float32
    bf16 = mybir.dt.bfloat16
    nc = bacc.Bacc("TRN2", target_bir_lowering=False, debug=False,
                   num_devices=NCORES)

    # per-(stream, tile) k-block window [kbase, kbase+kcnt) over h blocks
    kwin = _cache["kwin"]                  # list of NST*QT (kbase, kcnt)
    bases = np.cumsum([0] + [kc for _, kc in kwin]).tolist()
    noh = bases[-1]                        # total one-hot [128,128] tiles

    xh = nc.dram_tensor("xh", [128, SCH * KB * SCHW], bf16,
                        kind="ExternalInput").ap()
    wh = nc.dram_tensor("wh", [128, KB * P], bf16, kind="ExternalInput").ap()
    bias = nc.dram_tensor("bias", [128, P], f32, kind="ExternalInput").ap()
    oh = nc.dram_tensor("oh", [128, noh * 128], bf16,
                        kind="ExternalInput").ap()
    r1 = nc.dram_tensor("r1", [QCAP, 2 * P], bf16, kind="ExternalOutput").ap()
    r2 = nc.dram_tensor("r2", [QCAP, 2 * P], bf16, kind="ExternalOutput").ap()

    with tile.TileContext(nc) as tc:
        with (
            tc.tile_pool(name="consts", bufs=1) as consts,
            tc.tile_pool(name="xin", bufs=SCH) as xt_pool,
            tc.tile_pool(name="gout", bufs=6) as g_pool,
            tc.tile_pool(name="ps", bufs=4, space="PSUM") as ps_pool,
            tc.tile_pool(name="psg", bufs=4, space="PSUM") as psg_pool,
        ):
            identity = consts.tile([128, 128], bf16)
            make_identity(nc, identity)
            for i in range(NWARM):
                warm_ps = psg_pool.tile([128, 128], f32, tag="gps")
                nc.tensor.matmul(warm_ps, identity, identity,
                                 start=True, stop=True)

            w_sb = consts.tile([128, KB, P], bf16)
            nc.scalar.dma_start(w_sb,
                                wh.rearrange("p (kb j) -> p kb j", kb=KB))
            bias_sb = consts.tile([128, P], f32)
            nc.scalar.dma_start(bias_sb, bias)

            # x chunk loads, k-halves split across the two queues, up-front
            xh_view = xh.rearrange("p (c kb s) -> p c kb s", c=SCH, kb=KB)
            KH = KB // 2
            xt_tiles = [xt_pool.tile([128, KB, SCHW], bf16, name=f"xt{c}")
                        for c in range(SCH)]
            for c in range(SCH):
                nc.sync.dma_start(xt_tiles[c][:, 0:KH, :],
                                  xh_view[:, c, 0:KH, :])
            for c in range(SCH):
                nc.scalar.dma_start(xt_tiles[c][:, KH:KB, :],
                                    xh_view[:, c, KH:KB, :])
            # one-hot tiles, halves on each queue, behind the x loads
            oh_sb = consts.tile([128, noh, 128], bf16)
            oh_view = oh.rearrange("p (i q) -> p i q", q=128)
            # sync ring carries no weights, so it takes the larger oh share
            nh = (noh * 5) // 8
            nc.sync.dma_start(oh_sb[:, 0:nh, :], oh_view[:, 0:nh, :])
            nc.scalar.dma_start(oh_sb[:, nh:noh, :], oh_view[:, nh:noh, :])

            # phase 1: h = X @ W + b, blocks stay resident in SBUF
            h_tiles = []
            for c in range(SCH):
                for ml in range(SCHW // 128):
                    m = c * (SCHW // 128) + ml
                    h_ps = ps_pool.tile([128, P], f32, tag="hps")
                    for kb in range(KB):
                        nc.tensor.matmul(
                            h_ps,
                            xt_tiles[c][:, kb, ml * 128:(ml + 1) * 128],
                            w_sb[:, kb, :],
                            start=(kb == 0), stop=(kb == KB - 1))
                    h_sb = consts.tile([128, P], bf16, name=f"h{m}")
                    nc.vector.tensor_add(h_sb, h_ps, bias_sb)
                    h_tiles.append(h_sb)

            # phase 2: gather as one-hot matmuls
            for st in range(NST):
                r = r1 if st < 2 else r2
                endp = st % 2
                out_view = r.rearrange("(t p) c -> p t c", p=128)
                for t in range(QT):
                    j = st * QT + t
                    kbase, kcnt = kwin[j]
                    g_ps = psg_pool.tile([128, P], f32, tag="gps")
                    for l in range(kcnt):
                        nc.tensor.matmul(
                            g_ps, oh_sb[:, bases[j] + l, :],
                            h_tiles[kbase + l],
                            start=(l == 0), stop=(l == kcnt - 1))
                    g_sb = g_pool.tile([128, 1, P], bf16, tag="g")
                    if (st * QT + t) % 2 == 0:
                        nc.vector.tensor_copy(g_sb[:, 0, :], g_ps)
                    else:
                        nc.scalar.copy(g_sb[:, 0, :], g_ps)
                    eng = nc.sync if st < 2 else nc.scalar
                    eng.dma_start(
                        out_view[:, t:t + 1, endp * P:(endp + 1) * P],
                        g_sb)

    nc.compile()
    return nc


def _get_nc(kwin):
    key = ("nc", tuple(kwin))
    if key not in _cache:
        _cache["kwin"] = list(kwin)
        _cache[key] = _build_nc()
    return _cache[key]


def _numpy_ref(flag, encoded_input, start_ids_1, end_ids_1, query_batch_idx,
               start_ids_2, end_ids_2, W, b):
    h = encoded_input.astype(np.float32) @ W.astype(np.float32) + \
        b.astype(np.float32)
    qb = np.asarray(query_batch_idx).astype(np.int64)

    def span(s, e):
        s = np.asarray(s).astype(np.int64)
        e = np.asarray(e).astype(np.int64)
        rep = np.concatenate([h[qb, s], h[qb, e]], axis=-1)
        return rep * (e >= s)[:, None].astype(rep.dtype)

    return span(start_ids_1, end_ids_1), span(start_ids_2, end_ids_2)


def kernel(flag, encoded_input, start_ids_1, end_ids_1, query_batch_idx,
           start_ids_2, end_ids_2, W, b):
    import ml_dtypes
    from concourse.bass_utils import run_bass_kernel_spmd

    bf16 = ml_dtypes.bfloat16
    x_full = np.asarray(encoded_input, dtype=np.float32)
    w_np = np.asarray(W, dtype=np.float32)
    b_np = np.asarray(b).astype(np.float32)
    qb = np.asarray(query_batch_idx).astype(np.int64)
    s1 = np.asarray(start_ids_1).astype(np.int64)
    e1 = np.asarray(end_ids_1).astype(np.int64)
    s2 = np.asarray(start_ids_2).astype(np.int64)
    e2 = np.asarray(end_ids_2).astype(np.int64)

    in_range = (qb.min() >= 0 and qb.max() < B and
                all(a.min() >= 0 and a.max() < S for a in (s1, e1, s2, e2)))

    percore = []
    try:
        if not in_range or x_full.shape != (B, S, D):
            raise ValueError("shape/range")
        for bb in range(B):
            sel = qb == bb
            ids1 = np.nonzero(sel & (e1 >= s1))[0]
            ids2 = np.nonzero(sel & (e2 >= s2))[0]
            if len(ids1) > QCAP or len(ids2) > QCAP:
                raise ValueError("capacity overflow")
            rows = np.unique(np.concatenate(
                [s1[ids1], e1[ids1], s2[ids2], e2[ids2]]))
            if len(rows) > HROWS:
                raise ValueError("row overflow")
            # per endpoint-stream: slot ids sorted by referenced row
            streams = []
            for ids, a in [(ids1, s1), (ids1, e1), (ids2, s2), (ids2, e2)]:
                cr = np.searchsorted(rows, a[ids]).astype(np.int64)
                o = np.argsort(cr, kind="stable")
                streams.append((ids[o], cr[o]))
            percore.append((rows, streams))
        # merged k-windows per (stream, tile) across cores
        kwin = []
        for st in range(NST):
            for t in range(QT):
                lo, hi = HB - 1, 0
                for bb in range(B):
                    cr = percore[bb][1][st][1]
                    seg = cr[t * 128:(t + 1) * 128]
                    if len(seg):
                        lo = min(lo, int(seg[0]) // 128)
                        hi = max(hi, int(seg[-1]) // 128)
                if hi < lo:
                    lo, hi = 0, 0
                kwin.append((lo, hi - lo + 1))
        bases = np.cumsum([0] + [kc for _, kc in kwin])
        noh = int(bases[-1])

        wh = np.ascontiguousarray(
            w_np.reshape(KB, 128, P).transpose(1, 0, 2).reshape(128, KB * P)
        ).astype(bf16)
        bias_rep = np.ascontiguousarray(
            np.broadcast_to(b_np[None, :], (128, P)), dtype=np.float32)
        in_maps, ids_all = [], []
        for bb in range(B):
            rows, streams = percore[bb]
            ids_all.append(streams)
            oh_np = np.zeros((128, noh, 128), np.float32)
            for st in range(NST):
                ids, cr = streams[st]
                n = len(ids)
                for t in range(QT):
                    j = st * QT + t
                    kbase, kcnt = kwin[j]
                    seg = cr[t * 128:min(n, (t + 1) * 128)]
                    q = np.arange(len(seg))
                    oh_np[seg % 128, bases[j] + seg // 128 - kbase, q] = 1.0
            xc = np.zeros((HROWS, D), np.float32)
            xc[:len(rows)] = x_full[bb][rows]
            xr = xc.reshape(SCH, SCHW, KB, 128).transpose(3, 0, 2, 1) \
                .reshape(128, SCH * KB * SCHW)
            in_maps.append({
                "xh": np.ascontiguousarray(xr).astype(bf16),
                "wh": wh,
                "bias": bias_rep,
                "oh": np.ascontiguousarray(
                    oh_np.reshape(128, noh * 128)).astype(bf16),
            })
    except ValueError:
        res1, res2 = _numpy_ref(flag, x_full, s1, e1, qb, s2, e2, w_np, b_np)
        return np.asarray(res1, np.float32), np.asarray(res2, np.float32)

    nc = _get_nc(tuple(kwin))
    out = run_bass_kernel_spmd(nc, in_maps, core_ids=list(range(NCORES)))
    _cache["last_run"] = out

    res1 = np.zeros((NQ, 2 * P), np.float32)
    res2 = np.zeros((NQ, 2 * P), np.float32)
    for bb in range(B):
        streams = ids_all[bb]
        rr1 = np.asarray(out.results[bb]["r1"]).astype(np.float32)
        rr2 = np.asarray(out.results[bb]["r2"]).astype(np.float32)
        for st, (res, rr) in enumerate([(res1, rr1), (res1, rr1),
                                        (res2, rr2), (res2, rr2)]):
            ids, _ = streams[st]
            endp = st % 2
            n = len(ids)
            if n:
                res[ids, endp * P:(endp + 1) * P] = \
                    rr[:n, endp * P:(endp + 1) * P]
    return res1, res2
